# revision 3
# baseline (speedup 1.0000x reference)
"""Contrastive loss on Trainium2, data-parallel over the pair dimension.

Strategy (8 NeuronCores):
  - Shard the 32768 pairs across 8 cores (4096 pairs each); replicate the
    embedding table in each core's HBM.
  - Per core: indirect-DMA gather the a-rows and b-rows of its pairs from
    the table into SBUF in big chunks (1024 rows / 2 MiB per gather), then
    per 128-pair column: delta = a - b on the vector engine, and
    sum(delta^2) via a Square activation with free-dim accumulation on the
    scalar engine.  Small elementwise epilogue computes
    (1-same)*relu(1-d)^2 + same*d^2 and a per-partition reduction.
  - Each core returns 128 partial sums; the host adds them up and divides
    by (P + 1e-10).
"""

import numpy as np

N, D = 65536, 512          # embedding table
P_TOTAL = 32768            # number of pairs
NCORES = 8
M_MARGIN = 1.0
PARTS = 128                # SBUF partitions
PC = P_TOTAL // NCORES     # 4096 pairs per core
TJ = PC // PARTS           # 32 pair-columns per core
CH = 8                     # pair-columns per indirect-DMA gather chunk

_NC_CACHE = {}


def _build_nc(n=N, d=D, tj=TJ, ch=CH):
    import concourse.bass as bass
    import concourse.mybir as mybir
    from concourse import bacc
    from concourse.tile import TileContext

    f32 = mybir.dt.float32
    i32 = mybir.dt.int32
    AF = mybir.ActivationFunctionType
    OP = mybir.AluOpType

    # Bacc (not plain Bass): its compile pipeline runs
    # generate_event_semaphores, which splits multi-sem waits — TRN2
    # instructions encode at most one sync wait.
    nc = bacc.Bacc(None)
    emb_d = nc.declare_dram_parameter("emb", [n, d], f32, isOutput=False)
    idxa_d = nc.declare_dram_parameter("idx_a", [PARTS, tj], i32, isOutput=False)
    idxb_d = nc.declare_dram_parameter("idx_b", [PARTS, tj], i32, isOutput=False)
    same_d = nc.declare_dram_parameter("same", [PARTS, tj], f32, isOutput=False)
    out_d = nc.declare_dram_parameter("partial", [PARTS, 1], f32, isOutput=True)

    with TileContext(nc) as tc:
        with (
            tc.tile_pool(name="big", bufs=2) as big,
            tc.tile_pool(name="small", bufs=1) as small,
        ):
            idxa = small.tile([PARTS, tj], i32)
            idxb = small.tile([PARTS, tj], i32)
            same = small.tile([PARTS, tj], f32)
            sq = small.tile([PARTS, tj], f32)
            nc.sync.dma_start(idxa[:], idxa_d[:])
            nc.sync.dma_start(idxb[:], idxb_d[:])
            nc.sync.dma_start(same[:], same_d[:])

            nchunks = (tj + ch - 1) // ch
            for c in range(nchunks):
                c0 = c * ch
                cw = min(ch, tj - c0)
                a_t = big.tile([PARTS, cw * d], f32, tag="a")
                b_t = big.tile([PARTS, cw * d], f32, tag="b")
                nc.gpsimd.indirect_dma_start(
                    out=a_t[:],
                    out_offset=None,
                    in_=emb_d[:],
                    in_offset=bass.IndirectOffsetOnAxis(
                        ap=idxa[:, c0 : c0 + cw], axis=0
                    ),
                )
                nc.gpsimd.indirect_dma_start(
                    out=b_t[:],
                    out_offset=None,
                    in_=emb_d[:],
                    in_offset=bass.IndirectOffsetOnAxis(
                        ap=idxb[:, c0 : c0 + cw], axis=0
                    ),
                )
                for k in range(cw):
                    col = c0 + k
                    delta = big.tile([PARTS, d], f32, tag="delta")
                    junk = big.tile([PARTS, d], f32, tag="junk")
                    nc.vector.tensor_tensor(
                        out=delta[:],
                        in0=a_t[:, k * d : (k + 1) * d],
                        in1=b_t[:, k * d : (k + 1) * d],
                        op=OP.subtract,
                    )
                    # junk gets delta^2; accum_out gets sum over the free dim
                    nc.scalar.activation(
                        out=junk[:],
                        in_=delta[:],
                        func=AF.Square,
                        accum_out=sq[:, col : col + 1],
                    )

            # Epilogue on [PARTS, tj]:
            #   d2c  = max(sq, 1e-12)         (clip)
            #   dd   = sqrt(d2c)
            #   hin  = relu(m - dd)
            #   loss = (1-same)*hin^2 + same*d2c = hin^2 + same*(d2c - hin^2)
            d2c = small.tile([PARTS, tj], f32)
            nc.vector.tensor_scalar_max(d2c[:], sq[:], 1e-12)
            dd = small.tile([PARTS, tj], f32)
            nc.scalar.activation(out=dd[:], in_=d2c[:], func=AF.Sqrt)
            hin = small.tile([PARTS, tj], f32)
            nc.scalar.activation(
                out=hin[:], in_=dd[:], func=AF.Relu, scale=-1.0, bias=float(M_MARGIN)
            )
            h2 = small.tile([PARTS, tj], f32)
            nc.vector.tensor_tensor(out=h2[:], in0=hin[:], in1=hin[:], op=OP.mult)
            dif = small.tile([PARTS, tj], f32)
            nc.vector.tensor_tensor(out=dif[:], in0=d2c[:], in1=h2[:], op=OP.subtract)
            sd = small.tile([PARTS, tj], f32)
            nc.vector.tensor_tensor(out=sd[:], in0=same[:], in1=dif[:], op=OP.mult)
            lt = small.tile([PARTS, tj], f32)
            nc.vector.tensor_tensor(out=lt[:], in0=h2[:], in1=sd[:], op=OP.add)
            part = small.tile([PARTS, 1], f32)
            nc.vector.reduce_sum(part[:], lt[:], axis=mybir.AxisListType.X)
            nc.sync.dma_start(out_d[:], part[:])

    if not nc.is_finalized():
        nc.finalize()
    return nc


def _get_nc():
    key = (N, D, TJ, CH)
    if key not in _NC_CACHE:
        _NC_CACHE[key] = _build_nc()
    return _NC_CACHE[key]


def _make_in_maps(emb, pa, pb, ps):
    """Per-core input dicts; pair shard c is pairs [c*PC, (c+1)*PC)."""
    emb = np.ascontiguousarray(emb, dtype=np.float32)
    pa = np.ascontiguousarray(np.asarray(pa).astype(np.int32))
    pb = np.ascontiguousarray(np.asarray(pb).astype(np.int32))
    ps = np.ascontiguousarray(np.asarray(ps).astype(np.float32))
    in_maps = []
    for c in range(NCORES):
        sl = slice(c * PC, (c + 1) * PC)
        in_maps.append(
            {
                "emb": emb,
                "idx_a": pa[sl].reshape(PARTS, TJ),
                "idx_b": pb[sl].reshape(PARTS, TJ),
                "same": ps[sl].reshape(PARTS, TJ),
            }
        )
    return in_maps


def kernel(**inputs):
    from concourse.bass_utils import run_bass_kernel_spmd

    in_maps = _make_in_maps(
        inputs["embeddings"], inputs["pair_a"], inputs["pair_b"], inputs["pair_same"]
    )
    nc = _get_nc()
    res = run_bass_kernel_spmd(nc, in_maps, list(range(NCORES))).results
    total = 0.0
    for r in res:
        total += float(r["partial"].astype(np.float64).sum())
    return np.float32(total / (P_TOTAL + 1e-10))
